# revision 5
# baseline (speedup 1.0000x reference)
"""Trainium2 Bass kernel for nn_Density_Block (histogram_binning).

Math:  out_row = lerp(softmax(x@W + b)[L], softmax(...)[U], inter)
where U = ceil(t*255), L = max(U-1, 0), inter = 1 - (U - t*255).

Strategy:
  * Host: compute L/U/inter exactly as the reference (f32 ops), globally
    sort rows by L, deal sorted rows round-robin to the 8 cores so that
    local tile j on every core draws from the same global window of 1024
    sorted rows.  Within a window L spans a tiny range, so the two-point
    gather collapses to a dot product with a narrow per-tile weight strip
    (omega) whose column offsets are core-invariant (bakeable into the
    single SPMD program).
  * Device per 128-row tile: z = xT_tile.T @ W (+ ones^T @ bias, K=1
    accumulate) on PE in float32r (full-rate fp32); e = exp(z) on ScalarE
    batched 8 tiles per instruction; denominator s = sum(e) via
    tensor_scalar accum; numerator r via scalar_tensor_tensor against the
    omega strip; result = r * reciprocal(s) batched per 64 tiles.
  * Host: un-permute the per-core results back to the original row order.
"""

import numpy as np

N_CORES = 8
IND = 128
G = 256  # grid points (num_grid + 1)
TILE = 128  # rows per tile (PSUM/SBUF partition count)
MACRO_TILES = 8  # tiles per exp/psum macro ([128, 2048] = 4 PSUM banks)
BLOCK_MACROS = 8  # macros per finalize block (64 tiles)

_build_cache = {}


def _build(per_rows, c0, wid, off):
    import concourse.bass as bass
    import concourse.bacc as bacc
    import concourse.mybir as mybir
    import concourse.tile as tile

    F32 = mybir.dt.float32
    F32R = mybir.dt.float32r
    AOP = mybir.AluOpType
    AFT = mybir.ActivationFunctionType

    ntiles = per_rows // TILE
    om_total = int(off[-1])
    jw = int(max(wid))  # widest strip

    nc = bacc.Bacc("TRN2", target_bir_lowering=False, debug=False)
    xT_d = nc.dram_tensor("xT", [IND, per_rows], F32R, kind="ExternalInput")
    om_d = nc.dram_tensor("om", [TILE, om_total], F32, kind="ExternalInput")
    w_d = nc.dram_tensor("w", [IND, G], F32R, kind="ExternalInput")
    b_d = nc.dram_tensor("b", [1, G], F32R, kind="ExternalInput")
    res_d = nc.dram_tensor("res", [TILE, ntiles], F32, kind="ExternalOutput")

    MT = MACRO_TILES
    n_macros = ntiles // MT
    BM = min(BLOCK_MACROS, n_macros)
    n_blocks = n_macros // BM

    with tile.TileContext(nc) as tc:
        with (
            tc.tile_pool(name="const", bufs=1) as constp,
            tc.tile_pool(name="xt", bufs=3) as xtp,
            tc.tile_pool(name="psum", bufs=2, space=bass.MemorySpace.PSUM) as psump,
            tc.tile_pool(name="e", bufs=3) as ep,
            tc.tile_pool(name="junk", bufs=2) as junkp,
            tc.tile_pool(name="cols", bufs=2) as colsp,
        ):
            w_sb = constp.tile([IND, G], F32R)
            nc.sync.dma_start(out=w_sb[:], in_=w_d[:])
            b_sb = constp.tile([1, G], F32R)
            nc.sync.dma_start(out=b_sb[:], in_=b_d[:])
            om_sb = constp.tile([TILE, om_total], F32)
            nc.sync.dma_start(out=om_sb[:], in_=om_d[:])
            ones_f32 = constp.tile([1, IND], F32)
            nc.vector.memset(ones_f32[:], 1.0)
            ones_sb = ones_f32[:].bitcast(F32R)

            for blk in range(n_blocks):
                bt = BM * MT  # tiles per block
                r_cols = colsp.tile([TILE, bt], F32, tag="r_cols")
                s_cols = colsp.tile([TILE, bt], F32, tag="s_cols")
                for m in range(BM):
                    mac = blk * BM + m
                    xt_sb = xtp.tile([IND, MT * TILE], F32R, tag="xt")
                    nc.sync.dma_start(
                        out=xt_sb[:],
                        in_=xT_d[:, mac * MT * TILE : (mac + 1) * MT * TILE],
                    )
                    z_ps = psump.tile([TILE, MT * G], F32, tag="z")
                    for j in range(MT):
                        zsl = z_ps[:, j * G : (j + 1) * G]
                        nc.tensor.matmul(
                            zsl,
                            xt_sb[:, j * TILE : (j + 1) * TILE],
                            w_sb[:],
                            start=True,
                            stop=False,
                        )
                        nc.tensor.matmul(
                            zsl, ones_sb, b_sb[:], start=False, stop=True
                        )
                    e_sb = ep.tile([TILE, MT * G], F32, tag="e")
                    nc.scalar.activation(e_sb[:], z_ps[:], AFT.Exp)
                    for j in range(MT):
                        tj = mac * MT + j  # tile index
                        ci = m * MT + j  # column within block
                        junk_s = junkp.tile([TILE, G], F32, tag="js")
                        nc.vector.tensor_scalar(
                            junk_s[:],
                            e_sb[:, j * G : (j + 1) * G],
                            1.0,
                            None,
                            op0=AOP.mult,
                            op1=AOP.add,
                            accum_out=s_cols[:, ci : ci + 1],
                        )
                        wj = int(wid[tj])
                        oj = int(off[tj])
                        cj = int(c0[tj])
                        junk_r = junkp.tile([TILE, jw], F32, tag="jr")
                        nc.vector.scalar_tensor_tensor(
                            junk_r[:, :wj],
                            om_sb[:, oj : oj + wj],
                            0.0,
                            e_sb[:, j * G + cj : j * G + cj + wj],
                            op0=AOP.bypass,
                            op1=AOP.mult,
                            accum_out=r_cols[:, ci : ci + 1],
                        )
                sinv = colsp.tile([TILE, bt], F32, tag="sinv")
                nc.vector.reciprocal(sinv[:], s_cols[:])
                res_sb = colsp.tile([TILE, bt], F32, tag="res")
                nc.vector.tensor_tensor(
                    res_sb[:], r_cols[:], sinv[:], op=AOP.mult
                )
                nc.sync.dma_start(
                    out=res_d[:, blk * bt : (blk + 1) * bt], in_=res_sb[:]
                )
    nc.compile()
    return nc


def _host_prep(t, x, weight, bias, n_cores):
    t = np.asarray(t, dtype=np.float32).reshape(-1)
    x = np.asarray(x, dtype=np.float32)
    weight = np.asarray(weight, dtype=np.float32)
    bias = np.asarray(bias, dtype=np.float32).reshape(1, -1)
    n = t.shape[0]
    per = n // n_cores
    num_grid = weight.shape[1] - 1

    # Replicate the reference's fp32 index math exactly.
    tt = t * np.float32(num_grid)
    U = np.ceil(tt)
    inter = np.float32(1.0) - (U - tt)
    L = U - np.float32(1.0)
    L = L + (L < 0).astype(np.float32)
    Li = L.astype(np.int32)
    Ui = U.astype(np.int32)

    order = np.argsort(Li, kind="stable").astype(np.int64)
    wrows = n_cores * TILE  # global window rows per local tile
    ntiles = per // TILE
    Lw = Li[order].reshape(ntiles, wrows)
    Uw = Ui[order].reshape(ntiles, wrows)
    c0 = Lw.min(axis=1).astype(np.int64)
    hi = Uw.max(axis=1).astype(np.int64)
    wid = hi - c0 + 1
    off = np.zeros(ntiles + 1, np.int64)
    off[1:] = np.cumsum(wid)

    in_maps = []
    idx_list = []
    k = np.arange(per)
    p = k % TILE
    j = k // TILE
    for c in range(n_cores):
        idx = order[c::n_cores]
        xT = np.ascontiguousarray(x[idx].T)
        Lc = Li[idx].astype(np.int64)
        Uc = Ui[idx].astype(np.int64)
        ic = inter[idx]
        om = np.zeros((TILE, int(off[-1])), np.float32)
        np.add.at(om, (p, off[j] + (Lc - c0[j])), np.float32(1.0) - ic)
        np.add.at(om, (p, off[j] + (Uc - c0[j])), ic)
        in_maps.append({"xT": xT, "om": om, "w": weight, "b": bias})
        idx_list.append(idx)
    return per, c0, wid, off, in_maps, idx_list, n


def _ensure_ntff_hook():
    """Synthesize antenv.axon_hooks (absent in this container) so that
    run_bass_kernel_spmd(trace=True) can drive NRT profiling via the
    axon pjrt .so, and neutralize the artifact upload (no bucket here)."""
    import sys
    import types

    try:
        import antenv.axon_hooks  # noqa: F401
    except ImportError:
        import antenv
        from trn_agent_boot.trn_boot import _ntff_profile_via_ctypes

        mod = types.ModuleType("antenv.axon_hooks")
        state = {"hook": None}
        mod.set_axon_ntff_profile_hook = lambda h: state.__setitem__("hook", h)
        mod.get_axon_ntff_profile_hook = lambda: state["hook"]
        sys.modules["antenv.axon_hooks"] = mod
        antenv.axon_hooks = mod
        mod.set_axon_ntff_profile_hook(
            _ntff_profile_via_ctypes("/opt/axon/libaxon_pjrt.so")
        )
    import concourse.bass_utils as bu

    bu.upload_artifacts = lambda tmpdir: tmpdir


def run(t, x, weight, bias, trace=False):
    from concourse.bass_utils import run_bass_kernel_spmd

    if trace:
        _ensure_ntff_hook()

    per, c0, wid, off, in_maps, idx_list, n = _host_prep(
        t, x, weight, bias, N_CORES
    )
    key = (per, c0.tobytes(), wid.tobytes())
    nc = _build_cache.get(key)
    if nc is None:
        nc = _build(per, c0, wid, off)
        _build_cache[key] = nc
    rr = run_bass_kernel_spmd(nc, in_maps, list(range(N_CORES)), trace=trace)
    out = np.empty(n, np.float32)
    for c in range(N_CORES):
        out[idx_list[c]] = rr.results[c]["res"].T.reshape(per)
    return out, rr


def kernel(t, x, weight, bias):
    return run(t, x, weight, bias, trace=False)[0]


# revision 7
# speedup vs baseline: 2.0527x; 2.0527x over previous
"""Trainium2 Bass kernel for nn_Density_Block (histogram_binning).

Math:  out_row = lerp(softmax(x@W + b)[L], softmax(...)[U], inter)
where U = ceil(t*255), L = max(U-1, 0), inter = 1 - (U - t*255).

Strategy:
  * Host: compute L/U/inter exactly as the reference (f32 ops), globally
    sort rows by L, deal sorted rows round-robin to the 8 cores so that
    local tile j on every core draws from the same global window of 1024
    sorted rows.  Within a window L spans a tiny range, so the two-point
    gather collapses to a dot product with a narrow per-tile weight strip
    (omega) whose column offsets are core-invariant (bakeable into the
    single SPMD program).
  * Device per 128-row tile: z = xT_tile.T @ W (+ ones^T @ bias, K=1
    accumulate) on PE in float32r (full-rate fp32); e = exp(z) on ScalarE
    batched 8 tiles per instruction; denominator s = sum(e) via
    tensor_scalar accum; numerator r via scalar_tensor_tensor against the
    omega strip; result = r * reciprocal(s) batched per 64 tiles.
  * Host: un-permute the per-core results back to the original row order.
"""

import numpy as np

N_CORES = 8
IND = 128
G = 256  # grid points (num_grid + 1)
TILE = 128  # rows per tile (PSUM/SBUF partition count)
MACRO_TILES = 8  # tiles per exp/psum macro ([128, 2048] = 4 PSUM banks)
BLOCK_MACROS = 8  # macros per finalize block (64 tiles)

_build_cache = {}


N_ACT_ACCUM = 2  # tiles per macro whose denominator comes from ACT exp+accum


def _build(per_rows, c0, wid, off):
    import concourse.bass as bass
    import concourse.bacc as bacc
    import concourse.mybir as mybir
    import concourse.tile as tile

    F32 = mybir.dt.float32
    BF16 = mybir.dt.bfloat16
    AOP = mybir.AluOpType
    AFT = mybir.ActivationFunctionType

    ntiles = per_rows // TILE
    om_total = int(off[-1])
    jw = int(max(wid))  # widest strip

    nc = bacc.Bacc("TRN2", target_bir_lowering=False, debug=False)
    xh_d = nc.dram_tensor("xh", [IND, per_rows], BF16, kind="ExternalInput")
    xl_d = nc.dram_tensor("xl", [IND, per_rows], BF16, kind="ExternalInput")
    om_d = nc.dram_tensor("om", [TILE, om_total], F32, kind="ExternalInput")
    wh_d = nc.dram_tensor("wh", [IND, G], BF16, kind="ExternalInput")
    wl_d = nc.dram_tensor("wl", [IND, G], BF16, kind="ExternalInput")
    # bias split [bh; bl] repeated twice along columns -> [2, 2*G]
    b2_d = nc.dram_tensor("b2", [2, 2 * G], BF16, kind="ExternalInput")
    res_d = nc.dram_tensor("res", [TILE, ntiles], F32, kind="ExternalOutput")

    MT = MACRO_TILES
    n_macros = ntiles // MT
    BM = min(BLOCK_MACROS, n_macros)
    n_blocks = n_macros // BM
    na = N_ACT_ACCUM if MT > N_ACT_ACCUM else 0
    nb = MT - na  # tiles in the batched exp

    with tile.TileContext(nc) as tc:
        with (
            tc.tile_pool(name="const", bufs=1) as constp,
            tc.tile_pool(name="xt", bufs=3) as xtp,
            tc.tile_pool(name="psum", bufs=2, space=bass.MemorySpace.PSUM) as psump,
            tc.tile_pool(name="e", bufs=3) as ep,
            tc.tile_pool(name="junk", bufs=2) as junkp,
            tc.tile_pool(name="cols", bufs=2) as colsp,
        ):
            wh_sb = constp.tile([IND, G], BF16)
            nc.sync.dma_start(out=wh_sb[:], in_=wh_d[:])
            wl_sb = constp.tile([IND, G], BF16)
            nc.sync.dma_start(out=wl_sb[:], in_=wl_d[:])
            b2_sb = constp.tile([2, 2 * G], BF16)
            nc.sync.dma_start(out=b2_sb[:], in_=b2_d[:])
            om_sb = constp.tile([TILE, om_total], F32)
            nc.sync.dma_start(out=om_sb[:], in_=om_d[:])
            ones2 = constp.tile([2, IND], BF16)
            nc.vector.memset(ones2[:], 1.0)

            for blk in range(n_blocks):
                bt = BM * MT  # tiles per block
                r_cols = colsp.tile([TILE, bt], F32, tag="r_cols")
                s_cols = colsp.tile([TILE, bt], F32, tag="s_cols")
                for m in range(BM):
                    mac = blk * BM + m
                    r0 = mac * MT * TILE
                    r1 = (mac + 1) * MT * TILE
                    xh_sb = xtp.tile([IND, MT * TILE], BF16, tag="xh")
                    nc.sync.dma_start(out=xh_sb[:], in_=xh_d[:, r0:r1])
                    xl_sb = xtp.tile([IND, MT * TILE], BF16, tag="xl")
                    nc.sync.dma_start(out=xl_sb[:], in_=xl_d[:, r0:r1])
                    z_ps = psump.tile([TILE, MT * G], F32, tag="z")
                    # bias init: one K=2 matmul per PSUM bank (N = 2*G = 512)
                    for k in range(MT * G // (2 * G)):
                        nc.tensor.matmul(
                            z_ps[:, k * 2 * G : (k + 1) * 2 * G],
                            ones2[:],
                            b2_sb[:],
                            start=True,
                            stop=False,
                            skip_group_check=True,
                        )
                    for j in range(MT):
                        zsl = z_ps[:, j * G : (j + 1) * G]
                        xh_t = xh_sb[:, j * TILE : (j + 1) * TILE]
                        xl_t = xl_sb[:, j * TILE : (j + 1) * TILE]
                        nc.tensor.matmul(
                            zsl, xh_t, wh_sb[:], start=False, stop=False,
                            skip_group_check=True,
                        )
                        nc.tensor.matmul(
                            zsl, xh_t, wl_sb[:], start=False, stop=False,
                            skip_group_check=True,
                        )
                        nc.tensor.matmul(
                            zsl, xl_t, wh_sb[:], start=False, stop=True,
                            skip_group_check=True,
                        )
                    e_sb = ep.tile([TILE, MT * G], F32, tag="e")
                    if nb:
                        nc.scalar.activation(
                            e_sb[:, : nb * G], z_ps[:, : nb * G], AFT.Exp
                        )
                    for j in range(MT):
                        ci = m * MT + j  # column within block
                        if j >= nb:  # ACT computes exp AND the row sum
                            nc.scalar.activation(
                                e_sb[:, j * G : (j + 1) * G],
                                z_ps[:, j * G : (j + 1) * G],
                                AFT.Exp,
                                accum_out=s_cols[:, ci : ci + 1],
                            )
                    for j in range(MT):
                        tj = mac * MT + j  # tile index
                        ci = m * MT + j
                        if j < nb:
                            junk_s = junkp.tile([TILE, G], F32, tag="js")
                            nc.vector.tensor_scalar(
                                junk_s[:],
                                e_sb[:, j * G : (j + 1) * G],
                                1.0,
                                None,
                                op0=AOP.mult,
                                op1=AOP.add,
                                accum_out=s_cols[:, ci : ci + 1],
                            )
                        wj = int(wid[tj])
                        oj = int(off[tj])
                        cj = int(c0[tj])
                        junk_r = junkp.tile([TILE, jw], F32, tag="jr")
                        nc.vector.scalar_tensor_tensor(
                            junk_r[:, :wj],
                            om_sb[:, oj : oj + wj],
                            0.0,
                            e_sb[:, j * G + cj : j * G + cj + wj],
                            op0=AOP.bypass,
                            op1=AOP.mult,
                            accum_out=r_cols[:, ci : ci + 1],
                        )
                sinv = colsp.tile([TILE, bt], F32, tag="sinv")
                nc.vector.reciprocal(sinv[:], s_cols[:])
                res_sb = colsp.tile([TILE, bt], F32, tag="res")
                nc.vector.tensor_tensor(
                    res_sb[:], r_cols[:], sinv[:], op=AOP.mult
                )
                nc.sync.dma_start(
                    out=res_d[:, blk * bt : (blk + 1) * bt], in_=res_sb[:]
                )
    nc.compile()
    return nc


def _host_prep(t, x, weight, bias, n_cores):
    t = np.asarray(t, dtype=np.float32).reshape(-1)
    x = np.asarray(x, dtype=np.float32)
    weight = np.asarray(weight, dtype=np.float32)
    bias = np.asarray(bias, dtype=np.float32).reshape(1, -1)
    n = t.shape[0]
    per = n // n_cores
    num_grid = weight.shape[1] - 1

    # Replicate the reference's fp32 index math exactly.
    tt = t * np.float32(num_grid)
    U = np.ceil(tt)
    inter = np.float32(1.0) - (U - tt)
    L = U - np.float32(1.0)
    L = L + (L < 0).astype(np.float32)
    Li = L.astype(np.int32)
    Ui = U.astype(np.int32)

    order = np.argsort(Li, kind="stable").astype(np.int64)
    wrows = n_cores * TILE  # global window rows per local tile
    ntiles = per // TILE
    Lw = Li[order].reshape(ntiles, wrows)
    Uw = Ui[order].reshape(ntiles, wrows)
    c0 = Lw.min(axis=1).astype(np.int64)
    hi = Uw.max(axis=1).astype(np.int64)
    wid = hi - c0 + 1
    off = np.zeros(ntiles + 1, np.int64)
    off[1:] = np.cumsum(wid)

    import ml_dtypes

    bf16 = ml_dtypes.bfloat16
    wh = weight.astype(bf16)
    wl = (weight - wh.astype(np.float32)).astype(bf16)
    bh = bias.astype(bf16)
    bl = (bias - bh.astype(np.float32)).astype(bf16)
    b2 = np.concatenate(
        [np.tile(bh, (1, 2)), np.tile(bl, (1, 2))], axis=0
    )  # [2, 2*G]

    in_maps = []
    idx_list = []
    k = np.arange(per)
    p = k % TILE
    j = k // TILE
    for c in range(n_cores):
        idx = order[c::n_cores]
        xT = np.ascontiguousarray(x[idx].T)
        xh = xT.astype(bf16)
        xl = (xT - xh.astype(np.float32)).astype(bf16)
        Lc = Li[idx].astype(np.int64)
        Uc = Ui[idx].astype(np.int64)
        ic = inter[idx]
        om = np.zeros((TILE, int(off[-1])), np.float32)
        np.add.at(om, (p, off[j] + (Lc - c0[j])), np.float32(1.0) - ic)
        np.add.at(om, (p, off[j] + (Uc - c0[j])), ic)
        in_maps.append(
            {"xh": xh, "xl": xl, "om": om, "wh": wh, "wl": wl, "b2": b2}
        )
        idx_list.append(idx)
    return per, c0, wid, off, in_maps, idx_list, n


def _ensure_ntff_hook():
    """Synthesize antenv.axon_hooks (absent in this container) so that
    run_bass_kernel_spmd(trace=True) can drive NRT profiling via the
    axon pjrt .so, and neutralize the artifact upload (no bucket here)."""
    import sys
    import types

    try:
        import antenv.axon_hooks  # noqa: F401
    except ImportError:
        import antenv
        from trn_agent_boot.trn_boot import _ntff_profile_via_ctypes

        mod = types.ModuleType("antenv.axon_hooks")
        state = {"hook": None}
        mod.set_axon_ntff_profile_hook = lambda h: state.__setitem__("hook", h)
        mod.get_axon_ntff_profile_hook = lambda: state["hook"]
        sys.modules["antenv.axon_hooks"] = mod
        antenv.axon_hooks = mod
        mod.set_axon_ntff_profile_hook(
            _ntff_profile_via_ctypes("/opt/axon/libaxon_pjrt.so")
        )
    import concourse.bass_utils as bu

    bu.upload_artifacts = lambda tmpdir: tmpdir


def run(t, x, weight, bias, trace=False):
    from concourse.bass_utils import run_bass_kernel_spmd

    if trace:
        _ensure_ntff_hook()

    per, c0, wid, off, in_maps, idx_list, n = _host_prep(
        t, x, weight, bias, N_CORES
    )
    key = (per, c0.tobytes(), wid.tobytes())
    nc = _build_cache.get(key)
    if nc is None:
        nc = _build(per, c0, wid, off)
        _build_cache[key] = nc
    rr = run_bass_kernel_spmd(nc, in_maps, list(range(N_CORES)), trace=trace)
    out = np.empty(n, np.float32)
    for c in range(N_CORES):
        out[idx_list[c]] = rr.results[c]["res"].T.reshape(per)
    return out, rr


def kernel(t, x, weight, bias):
    return run(t, x, weight, bias, trace=False)[0]


# revision 12
# speedup vs baseline: 2.4044x; 1.1714x over previous
"""Trainium2 Bass kernel for nn_Density_Block (histogram_binning).

Math:  out_row = lerp(softmax(x@W + b)[L], softmax(...)[U], inter)
where U = ceil(t*255), L = max(U-1, 0), inter = 1 - (U - t*255).

Strategy:
  * Host: compute L/U/inter exactly as the reference (f32 ops), globally
    sort rows by L, deal sorted rows round-robin to the 8 cores so that
    local tile j on every core draws from the same global window of 1024
    sorted rows.  Within a window L spans a tiny range, so the two-point
    gather collapses to a dot product with a narrow per-tile weight strip
    (omega) whose column offsets are core-invariant (bakeable into the
    single SPMD program).
  * Device per 128-row tile: z = xT_tile.T @ W (+ ones^T @ bias, K=1
    accumulate) on PE in float32r (full-rate fp32); e = exp(z) on ScalarE
    batched 8 tiles per instruction; denominator s = sum(e) via
    tensor_scalar accum; numerator r via scalar_tensor_tensor against the
    omega strip; result = r * reciprocal(s) batched per 64 tiles.
  * Host: un-permute the per-core results back to the original row order.
"""

import numpy as np

N_CORES = 8
IND = 128
G = 256  # grid points (num_grid + 1)
TILE = 128  # rows per tile (PSUM/SBUF partition count)
MACRO_TILES = 8  # tiles per exp/psum macro ([128, 2048] = 4 PSUM banks)
BLOCK_MACROS = 8  # macros per finalize block (64 tiles)

_build_cache = {}


N_ACT_ACCUM = 2  # tiles per macro whose denominator comes from ACT exp+accum


def _build(per_rows, c0, wid, off):
    import concourse.bass as bass
    import concourse.bacc as bacc
    import concourse.mybir as mybir
    import concourse.tile as tile

    F32 = mybir.dt.float32
    BF16 = mybir.dt.bfloat16
    AOP = mybir.AluOpType
    AFT = mybir.ActivationFunctionType

    ntiles = per_rows // TILE
    om_total = int(off[-1])
    jw = int(max(wid))  # widest strip

    nc = bacc.Bacc("TRN2", target_bir_lowering=False, debug=False)
    xh_d = nc.dram_tensor("xh", [IND, per_rows], BF16, kind="ExternalInput")
    xl_d = nc.dram_tensor("xl", [IND, per_rows], BF16, kind="ExternalInput")
    om_d = nc.dram_tensor("om", [TILE, om_total], F32, kind="ExternalInput")
    wh_d = nc.dram_tensor("wh", [IND, G], BF16, kind="ExternalInput")
    wl_d = nc.dram_tensor("wl", [IND, G], BF16, kind="ExternalInput")
    # bias split [bh; bl] repeated twice along columns -> [2, 2*G]
    b2_d = nc.dram_tensor("b2", [2, 2 * G], BF16, kind="ExternalInput")
    # exp(bias) broadcast across partitions, for the bias-less tiles
    bb_d = nc.dram_tensor("bb", [TILE, G], F32, kind="ExternalInput")
    res_d = nc.dram_tensor("res", [TILE, ntiles], F32, kind="ExternalOutput")

    MT = MACRO_TILES
    n_macros = ntiles // MT
    BM = min(BLOCK_MACROS, n_macros)
    n_blocks = n_macros // BM
    na = N_ACT_ACCUM if MT > N_ACT_ACCUM else 0
    nb = MT - na  # tiles in the batched exp

    with tile.TileContext(nc) as tc:
        with (
            tc.tile_pool(name="const", bufs=1) as constp,
            tc.tile_pool(name="xt", bufs=3) as xtp,
            tc.tile_pool(name="psum", bufs=2, space=bass.MemorySpace.PSUM) as psump,
            tc.tile_pool(name="e", bufs=3) as ep,
            tc.tile_pool(name="junk", bufs=2) as junkp,
            tc.tile_pool(name="cols", bufs=2) as colsp,
        ):
            wh_sb = constp.tile([IND, G], BF16)
            nc.sync.dma_start(out=wh_sb[:], in_=wh_d[:])
            wl_sb = constp.tile([IND, G], BF16)
            nc.sync.dma_start(out=wl_sb[:], in_=wl_d[:])
            b2_sb = constp.tile([2, 2 * G], BF16)
            nc.sync.dma_start(out=b2_sb[:], in_=b2_d[:])
            om_sb = constp.tile([TILE, om_total], F32)
            nc.sync.dma_start(out=om_sb[:], in_=om_d[:])
            bb_sb = constp.tile([TILE, G], F32)
            nc.sync.dma_start(out=bb_sb[:], in_=bb_d[:])
            ones2 = constp.tile([2, IND], BF16)
            nc.vector.memset(ones2[:], 1.0)

            for blk in range(n_blocks):
                bt = BM * MT  # tiles per block
                r_cols = colsp.tile([TILE, bt], F32, tag="r_cols")
                s_cols = colsp.tile([TILE, bt], F32, tag="s_cols")
                for m in range(BM):
                    mac = blk * BM + m
                    r0 = mac * MT * TILE
                    r1 = (mac + 1) * MT * TILE
                    xh_sb = xtp.tile([IND, MT * TILE], BF16, tag="xh")
                    nc.sync.dma_start(out=xh_sb[:], in_=xh_d[:, r0:r1])
                    xl_sb = xtp.tile([IND, MT * TILE], BF16, tag="xl")
                    nc.sync.dma_start(out=xl_sb[:], in_=xl_d[:, r0:r1])
                    z_ps = psump.tile([TILE, MT * G], F32, tag="z")
                    # bias init only for the ACT-accum tiles (last na tiles):
                    # one K=2 matmul covering their 2*G columns (one bank)
                    if na:
                        nc.tensor.matmul(
                            z_ps[:, nb * G : MT * G],
                            ones2[:],
                            b2_sb[:],
                            start=True,
                            stop=False,
                            skip_group_check=True,
                        )
                    for j in range(MT):
                        zsl = z_ps[:, j * G : (j + 1) * G]
                        xh_t = xh_sb[:, j * TILE : (j + 1) * TILE]
                        xl_t = xl_sb[:, j * TILE : (j + 1) * TILE]
                        nc.tensor.matmul(
                            zsl, xh_t, wh_sb[:], start=(j < nb), stop=False,
                            skip_group_check=True,
                        )
                        nc.tensor.matmul(
                            zsl, xh_t, wl_sb[:], start=False, stop=False,
                            skip_group_check=True,
                        )
                        nc.tensor.matmul(
                            zsl, xl_t, wh_sb[:], start=False, stop=True,
                            skip_group_check=True,
                        )
                    e_sb = ep.tile([TILE, MT * G], F32, tag="e")
                    if nb:
                        nc.scalar.activation(
                            e_sb[:, : nb * G], z_ps[:, : nb * G], AFT.Exp
                        )
                    for j in range(MT):
                        ci = m * MT + j  # column within block
                        if j >= nb:  # ACT computes exp AND the row sum
                            nc.scalar.activation(
                                e_sb[:, j * G : (j + 1) * G],
                                z_ps[:, j * G : (j + 1) * G],
                                AFT.Exp,
                                accum_out=s_cols[:, ci : ci + 1],
                            )
                    for j in range(MT):
                        tj = mac * MT + j  # tile index
                        ci = m * MT + j
                        if j < nb:
                            junk_s = junkp.tile([TILE, G], F32, tag="js")
                            nc.vector.scalar_tensor_tensor(
                                junk_s[:],
                                e_sb[:, j * G : (j + 1) * G],
                                0.0,
                                bb_sb[:],
                                op0=AOP.bypass,
                                op1=AOP.mult,
                                accum_out=s_cols[:, ci : ci + 1],
                            )
                        wj = int(wid[tj])
                        oj = int(off[tj])
                        cj = int(c0[tj])
                        junk_r = junkp.tile([TILE, jw], F32, tag="jr")
                        nc.vector.scalar_tensor_tensor(
                            junk_r[:, :wj],
                            om_sb[:, oj : oj + wj],
                            0.0,
                            e_sb[:, j * G + cj : j * G + cj + wj],
                            op0=AOP.bypass,
                            op1=AOP.mult,
                            accum_out=r_cols[:, ci : ci + 1],
                        )
                sinv = colsp.tile([TILE, bt], F32, tag="sinv")
                nc.vector.reciprocal(sinv[:], s_cols[:])
                res_sb = colsp.tile([TILE, bt], F32, tag="res")
                nc.vector.tensor_tensor(
                    res_sb[:], r_cols[:], sinv[:], op=AOP.mult
                )
                nc.sync.dma_start(
                    out=res_d[:, blk * bt : (blk + 1) * bt], in_=res_sb[:]
                )
    nc.compile()
    return nc


def _host_prep(t, x, weight, bias, n_cores):
    t = np.asarray(t, dtype=np.float32).reshape(-1)
    x = np.asarray(x, dtype=np.float32)
    weight = np.asarray(weight, dtype=np.float32)
    bias = np.asarray(bias, dtype=np.float32).reshape(1, -1)
    n = t.shape[0]
    per = n // n_cores
    num_grid = weight.shape[1] - 1

    # Replicate the reference's fp32 index math exactly.
    tt = t * np.float32(num_grid)
    U = np.ceil(tt)
    inter = np.float32(1.0) - (U - tt)
    L = U - np.float32(1.0)
    L = L + (L < 0).astype(np.float32)
    Li = L.astype(np.int32)
    Ui = U.astype(np.int32)

    order = np.argsort(Li, kind="stable").astype(np.int64)
    wrows = n_cores * TILE  # global window rows per local tile
    ntiles = per // TILE
    Lw = Li[order].reshape(ntiles, wrows)
    Uw = Ui[order].reshape(ntiles, wrows)
    c0 = Lw.min(axis=1).astype(np.int64)
    hi = Uw.max(axis=1).astype(np.int64)
    wid = hi - c0 + 1
    off = np.zeros(ntiles + 1, np.int64)
    off[1:] = np.cumsum(wid)

    import ml_dtypes

    bf16 = ml_dtypes.bfloat16
    wh = weight.astype(bf16)
    wl = (weight - wh.astype(np.float32)).astype(bf16)
    bh = bias.astype(bf16)
    bl = (bias - bh.astype(np.float32)).astype(bf16)
    b2 = np.concatenate(
        [np.tile(bh, (1, 2)), np.tile(bl, (1, 2))], axis=0
    )  # [2, 2*G]
    beta = np.exp(bias.astype(np.float64)).astype(np.float32)[0]  # [G]
    bb = np.broadcast_to(beta, (TILE, beta.shape[0])).copy()

    na = N_ACT_ACCUM if MACRO_TILES > N_ACT_ACCUM else 0
    nb = MACRO_TILES - na

    in_maps = []
    idx_list = []
    k = np.arange(per)
    p = k % TILE
    j = k // TILE
    biasless = (j % MACRO_TILES) < nb  # rows whose e excludes the bias
    for c in range(n_cores):
        idx = order[c::n_cores]
        xT = np.ascontiguousarray(x[idx].T)
        xh = xT.astype(bf16)
        xl = (xT - xh.astype(np.float32)).astype(bf16)
        Lc = Li[idx].astype(np.int64)
        Uc = Ui[idx].astype(np.int64)
        ic = inter[idx]
        wL = np.float32(1.0) - ic
        wU = ic.copy()
        wL = np.where(biasless, wL * beta[Lc], wL).astype(np.float32)
        wU = np.where(biasless, wU * beta[Uc], wU).astype(np.float32)
        om = np.zeros((TILE, int(off[-1])), np.float32)
        np.add.at(om, (p, off[j] + (Lc - c0[j])), wL)
        np.add.at(om, (p, off[j] + (Uc - c0[j])), wU)
        in_maps.append(
            {
                "xh": xh,
                "xl": xl,
                "om": om,
                "wh": wh,
                "wl": wl,
                "b2": b2,
                "bb": bb,
            }
        )
        idx_list.append(idx)
    return per, c0, wid, off, in_maps, idx_list, n


def _ensure_ntff_hook():
    """Synthesize antenv.axon_hooks (absent in this container) so that
    run_bass_kernel_spmd(trace=True) can drive NRT profiling via the
    axon pjrt .so, and neutralize the artifact upload (no bucket here)."""
    import sys
    import types

    try:
        import antenv.axon_hooks  # noqa: F401
    except ImportError:
        import antenv
        from trn_agent_boot.trn_boot import _ntff_profile_via_ctypes

        mod = types.ModuleType("antenv.axon_hooks")
        state = {"hook": None}
        mod.set_axon_ntff_profile_hook = lambda h: state.__setitem__("hook", h)
        mod.get_axon_ntff_profile_hook = lambda: state["hook"]
        sys.modules["antenv.axon_hooks"] = mod
        antenv.axon_hooks = mod
        mod.set_axon_ntff_profile_hook(
            _ntff_profile_via_ctypes("/opt/axon/libaxon_pjrt.so")
        )
    import concourse.bass_utils as bu

    bu.upload_artifacts = lambda tmpdir: tmpdir


def run(t, x, weight, bias, trace=False):
    from concourse.bass_utils import run_bass_kernel_spmd

    if trace:
        _ensure_ntff_hook()

    per, c0, wid, off, in_maps, idx_list, n = _host_prep(
        t, x, weight, bias, N_CORES
    )
    key = (per, c0.tobytes(), wid.tobytes())
    nc = _build_cache.get(key)
    if nc is None:
        nc = _build(per, c0, wid, off)
        _build_cache[key] = nc
    rr = run_bass_kernel_spmd(nc, in_maps, list(range(N_CORES)), trace=trace)
    out = np.empty(n, np.float32)
    for c in range(N_CORES):
        out[idx_list[c]] = rr.results[c]["res"].T.reshape(per)
    return out, rr


def kernel(t, x, weight, bias):
    return run(t, x, weight, bias, trace=False)[0]


# revision 15
# speedup vs baseline: 2.4233x; 1.0078x over previous
"""Trainium2 Bass kernel for nn_Density_Block (histogram_binning).

Math:  out_row = lerp(softmax(x@W + b)[L], softmax(...)[U], inter)
where U = ceil(t*255), L = max(U-1, 0), inter = 1 - (U - t*255).

Strategy:
  * Host: compute L/U/inter exactly as the reference (f32 ops), globally
    sort rows by L, deal sorted rows round-robin to the 8 cores so that
    local tile j on every core draws from the same global window of 1024
    sorted rows.  Within a window L spans a tiny range, so the two-point
    gather collapses to a dot product with a narrow per-tile weight strip
    (omega) whose column offsets are core-invariant (bakeable into the
    single SPMD program).
  * Device per 128-row tile: z = xT_tile.T @ W (+ ones^T @ bias, K=1
    accumulate) on PE in float32r (full-rate fp32); e = exp(z) on ScalarE
    batched 8 tiles per instruction; denominator s = sum(e) via
    tensor_scalar accum; numerator r via scalar_tensor_tensor against the
    omega strip; result = r * reciprocal(s) batched per 64 tiles.
  * Host: un-permute the per-core results back to the original row order.
"""

import numpy as np

N_CORES = 8
IND = 128
G = 256  # grid points (num_grid + 1)
TILE = 128  # rows per tile (PSUM/SBUF partition count)
MACRO_TILES = 8  # tiles per exp/psum macro ([128, 2048] = 4 PSUM banks)
BLOCK_MACROS = 8  # macros per finalize block (64 tiles)

_build_cache = {}


N_ACT_ACCUM = 2  # tiles per macro whose denominator comes from ACT exp+accum


def _build(per_rows, c0, wid, off):
    import concourse.bass as bass
    import concourse.bacc as bacc
    import concourse.mybir as mybir
    import concourse.tile as tile

    F32 = mybir.dt.float32
    BF16 = mybir.dt.bfloat16
    AOP = mybir.AluOpType
    AFT = mybir.ActivationFunctionType

    ntiles = per_rows // TILE
    om_total = int(off[-1])
    jw = int(max(wid))  # widest strip

    nc = bacc.Bacc("TRN2", target_bir_lowering=False, debug=False)
    xh_d = nc.dram_tensor("xh", [IND, per_rows], BF16, kind="ExternalInput")
    xl_d = nc.dram_tensor("xl", [IND, per_rows], BF16, kind="ExternalInput")
    om_d = nc.dram_tensor("om", [TILE, om_total], F32, kind="ExternalInput")
    wh_d = nc.dram_tensor("wh", [IND, G], BF16, kind="ExternalInput")
    wl_d = nc.dram_tensor("wl", [IND, G], BF16, kind="ExternalInput")
    # bias split [bh; bl] repeated twice along columns -> [2, 2*G]
    b2_d = nc.dram_tensor("b2", [2, 2 * G], BF16, kind="ExternalInput")
    # exp(bias) broadcast across partitions, for the bias-less tiles
    bb_d = nc.dram_tensor("bb", [TILE, G], F32, kind="ExternalInput")
    res_d = nc.dram_tensor("res", [TILE, ntiles], F32, kind="ExternalOutput")

    MT = MACRO_TILES
    n_macros = ntiles // MT
    BM = min(BLOCK_MACROS, n_macros)
    n_blocks = n_macros // BM
    na = N_ACT_ACCUM if MT > N_ACT_ACCUM else 0
    nb = MT - na  # tiles in the batched exp

    with tile.TileContext(nc) as tc:
        with (
            tc.tile_pool(name="const", bufs=1) as constp,
            tc.tile_pool(name="xt", bufs=4) as xtp,
            tc.tile_pool(name="psum", bufs=2, space=bass.MemorySpace.PSUM) as psump,
            tc.tile_pool(name="e", bufs=4) as ep,
            tc.tile_pool(name="junk", bufs=2) as junkp,
            tc.tile_pool(name="cols", bufs=2) as colsp,
        ):
            wh_sb = constp.tile([IND, G], BF16)
            nc.sync.dma_start(out=wh_sb[:], in_=wh_d[:])
            wl_sb = constp.tile([IND, G], BF16)
            nc.sync.dma_start(out=wl_sb[:], in_=wl_d[:])
            b2_sb = constp.tile([2, 2 * G], BF16)
            nc.sync.dma_start(out=b2_sb[:], in_=b2_d[:])
            ones2 = constp.tile([2, IND], BF16)
            nc.vector.memset(ones2[:], 1.0)
            bb_sb = constp.tile([TILE, G], F32)
            nc.sync.dma_start(out=bb_sb[:], in_=bb_d[:])

            for blk in range(n_blocks):
                bt = BM * MT  # tiles per block
                o0 = int(off[blk * bt])
                o1 = int(off[(blk + 1) * bt])
                om_sb = xtp.tile([TILE, max(o1 - o0, 4)], F32, tag="om")
                nc.sync.dma_start(out=om_sb[:, : o1 - o0], in_=om_d[:, o0:o1])
                r_cols = colsp.tile([TILE, bt], F32, tag="r_cols")
                s_cols = colsp.tile([TILE, bt], F32, tag="s_cols")
                for m in range(BM):
                    mac = blk * BM + m
                    r0 = mac * MT * TILE
                    r1 = (mac + 1) * MT * TILE
                    xh_sb = xtp.tile([IND, MT * TILE], BF16, tag="xh")
                    nc.sync.dma_start(out=xh_sb[:], in_=xh_d[:, r0:r1])
                    xl_sb = xtp.tile([IND, MT * TILE], BF16, tag="xl")
                    nc.sync.dma_start(out=xl_sb[:], in_=xl_d[:, r0:r1])
                    z_ps = psump.tile([TILE, MT * G], F32, tag="z")
                    # accum tiles (j >= nb) first so their ACT work starts
                    # while PE is still on the batched tiles
                    jorder = list(range(nb, MT)) + list(range(nb))
                    # bias init only for the ACT-accum tiles (last na tiles):
                    # one K=2 matmul covering their 2*G columns (one bank)
                    if na:
                        nc.tensor.matmul(
                            z_ps[:, nb * G : MT * G],
                            ones2[:],
                            b2_sb[:],
                            start=True,
                            stop=False,
                            skip_group_check=True,
                        )
                    for j in jorder:
                        zsl = z_ps[:, j * G : (j + 1) * G]
                        xh_t = xh_sb[:, j * TILE : (j + 1) * TILE]
                        xl_t = xl_sb[:, j * TILE : (j + 1) * TILE]
                        nc.tensor.matmul(
                            zsl, xh_t, wh_sb[:], start=(j < nb), stop=False,
                            skip_group_check=True,
                        )
                        nc.tensor.matmul(
                            zsl, xh_t, wl_sb[:], start=False, stop=False,
                            skip_group_check=True,
                        )
                        nc.tensor.matmul(
                            zsl, xl_t, wh_sb[:], start=False, stop=True,
                            skip_group_check=True,
                        )
                    e_sb = ep.tile([TILE, MT * G], F32, tag="e")
                    for j in range(nb, MT):  # ACT computes exp AND row sum
                        ci = m * MT + j
                        nc.scalar.activation(
                            e_sb[:, j * G : (j + 1) * G],
                            z_ps[:, j * G : (j + 1) * G],
                            AFT.Exp,
                            accum_out=s_cols[:, ci : ci + 1],
                        )
                    if nb:
                        nc.scalar.activation(
                            e_sb[:, : nb * G], z_ps[:, : nb * G], AFT.Exp
                        )
                    for j in jorder:
                        tj = mac * MT + j  # tile index
                        ci = m * MT + j
                        if j < nb:
                            junk_s = junkp.tile([TILE, G], F32, tag="js")
                            nc.vector.scalar_tensor_tensor(
                                junk_s[:],
                                e_sb[:, j * G : (j + 1) * G],
                                0.0,
                                bb_sb[:],
                                op0=AOP.bypass,
                                op1=AOP.mult,
                                accum_out=s_cols[:, ci : ci + 1],
                            )
                        wj = int(wid[tj])
                        oj = int(off[tj]) - o0
                        cj = int(c0[tj])
                        junk_r = junkp.tile([TILE, jw], F32, tag="jr")
                        nc.vector.scalar_tensor_tensor(
                            junk_r[:, :wj],
                            om_sb[:, oj : oj + wj],
                            0.0,
                            e_sb[:, j * G + cj : j * G + cj + wj],
                            op0=AOP.bypass,
                            op1=AOP.mult,
                            accum_out=r_cols[:, ci : ci + 1],
                        )
                sinv = colsp.tile([TILE, bt], F32, tag="sinv")
                nc.vector.reciprocal(sinv[:], s_cols[:])
                res_sb = colsp.tile([TILE, bt], F32, tag="res")
                nc.vector.tensor_tensor(
                    res_sb[:], r_cols[:], sinv[:], op=AOP.mult
                )
                nc.sync.dma_start(
                    out=res_d[:, blk * bt : (blk + 1) * bt], in_=res_sb[:]
                )
    nc.compile()
    return nc


def _host_prep(t, x, weight, bias, n_cores):
    t = np.asarray(t, dtype=np.float32).reshape(-1)
    x = np.asarray(x, dtype=np.float32)
    weight = np.asarray(weight, dtype=np.float32)
    bias = np.asarray(bias, dtype=np.float32).reshape(1, -1)
    n = t.shape[0]
    per = n // n_cores
    num_grid = weight.shape[1] - 1

    # Replicate the reference's fp32 index math exactly.
    tt = t * np.float32(num_grid)
    U = np.ceil(tt)
    inter = np.float32(1.0) - (U - tt)
    L = U - np.float32(1.0)
    L = L + (L < 0).astype(np.float32)
    Li = L.astype(np.int32)
    Ui = U.astype(np.int32)

    order = np.argsort(Li, kind="stable").astype(np.int64)
    wrows = n_cores * TILE  # global window rows per local tile
    ntiles = per // TILE
    Lw = Li[order].reshape(ntiles, wrows)
    Uw = Ui[order].reshape(ntiles, wrows)
    c0 = Lw.min(axis=1).astype(np.int64)
    hi = Uw.max(axis=1).astype(np.int64)
    wid = hi - c0 + 1
    off = np.zeros(ntiles + 1, np.int64)
    off[1:] = np.cumsum(wid)

    import ml_dtypes

    bf16 = ml_dtypes.bfloat16
    wh = weight.astype(bf16)
    wl = (weight - wh.astype(np.float32)).astype(bf16)
    bh = bias.astype(bf16)
    bl = (bias - bh.astype(np.float32)).astype(bf16)
    b2 = np.concatenate(
        [np.tile(bh, (1, 2)), np.tile(bl, (1, 2))], axis=0
    )  # [2, 2*G]
    beta = np.exp(bias.astype(np.float64)).astype(np.float32)[0]  # [G]
    bb = np.broadcast_to(beta, (TILE, beta.shape[0])).copy()

    na = N_ACT_ACCUM if MACRO_TILES > N_ACT_ACCUM else 0
    nb = MACRO_TILES - na

    in_maps = []
    idx_list = []
    k = np.arange(per)
    p = k % TILE
    j = k // TILE
    biasless = (j % MACRO_TILES) < nb  # rows whose e excludes the bias
    for c in range(n_cores):
        idx = order[c::n_cores]
        xT = np.ascontiguousarray(x[idx].T)
        xh = xT.astype(bf16)
        xl = (xT - xh.astype(np.float32)).astype(bf16)
        Lc = Li[idx].astype(np.int64)
        Uc = Ui[idx].astype(np.int64)
        ic = inter[idx]
        wL = np.float32(1.0) - ic
        wU = ic.copy()
        wL = np.where(biasless, wL * beta[Lc], wL).astype(np.float32)
        wU = np.where(biasless, wU * beta[Uc], wU).astype(np.float32)
        om = np.zeros((TILE, int(off[-1])), np.float32)
        np.add.at(om, (p, off[j] + (Lc - c0[j])), wL)
        np.add.at(om, (p, off[j] + (Uc - c0[j])), wU)
        in_maps.append(
            {
                "xh": xh,
                "xl": xl,
                "om": om,
                "wh": wh,
                "wl": wl,
                "b2": b2,
                "bb": bb,
            }
        )
        idx_list.append(idx)
    return per, c0, wid, off, in_maps, idx_list, n


def _ensure_ntff_hook():
    """Synthesize antenv.axon_hooks (absent in this container) so that
    run_bass_kernel_spmd(trace=True) can drive NRT profiling via the
    axon pjrt .so, and neutralize the artifact upload (no bucket here)."""
    import sys
    import types

    try:
        import antenv.axon_hooks  # noqa: F401
    except ImportError:
        import antenv
        from trn_agent_boot.trn_boot import _ntff_profile_via_ctypes

        mod = types.ModuleType("antenv.axon_hooks")
        state = {"hook": None}
        mod.set_axon_ntff_profile_hook = lambda h: state.__setitem__("hook", h)
        mod.get_axon_ntff_profile_hook = lambda: state["hook"]
        sys.modules["antenv.axon_hooks"] = mod
        antenv.axon_hooks = mod
        mod.set_axon_ntff_profile_hook(
            _ntff_profile_via_ctypes("/opt/axon/libaxon_pjrt.so")
        )
    import concourse.bass_utils as bu

    bu.upload_artifacts = lambda tmpdir: tmpdir


def run(t, x, weight, bias, trace=False):
    from concourse.bass_utils import run_bass_kernel_spmd

    if trace:
        _ensure_ntff_hook()

    per, c0, wid, off, in_maps, idx_list, n = _host_prep(
        t, x, weight, bias, N_CORES
    )
    key = (per, c0.tobytes(), wid.tobytes())
    nc = _build_cache.get(key)
    if nc is None:
        nc = _build(per, c0, wid, off)
        _build_cache[key] = nc
    rr = run_bass_kernel_spmd(nc, in_maps, list(range(N_CORES)), trace=trace)
    out = np.empty(n, np.float32)
    for c in range(N_CORES):
        out[idx_list[c]] = rr.results[c]["res"].T.reshape(per)
    return out, rr


def kernel(t, x, weight, bias):
    return run(t, x, weight, bias, trace=False)[0]
